# revision 1
# baseline (speedup 1.0000x reference)
"""Trainium2 Bass kernel for RealVirtualAttention (masked segment-mean pool + HAN
semantic attention), SPMD across 8 NeuronCores.

Strategy (data-parallel over graphs):
  - 4096 graphs -> 64 blocks of 64 graphs; core i owns blocks 8i..8i+7
    (nodes sharded at graph boundaries, batch is sorted).
  - Per 128-node tile, build a one-hot selector [128 nodes, 128 cols] on
    DVE/GpSimd from host-precomputed column ids col = (batch - g_base) +
    64*(z==VIRTUAL), and matmul-accumulate selT.T @ X into PSUM -> per-block
    [128 rows = (graph, real/virt), 150] masked segment sums on TensorE.
  - Scale rows by host-precomputed 1/max(count,1) -> means; tiny HAN head
    (W1, tanh, q) per block; scores all-reduced (8 bytes) across cores for
    the global softmax beta; final combine via a beta-weighted selector
    matmul; each core writes its 512-graph slice of the output.
"""

import numpy as np

import concourse.bacc as bacc
import concourse.bass as bass
import concourse.tile as tile
import concourse.mybir as mybir
from concourse.bass_utils import run_bass_kernel_spmd

F32 = mybir.dt.float32
N_CORES = 8
B = 4096          # graphs
D = 150           # feature dim
A = 128           # attention hidden dim
GB = 64           # graphs per block (x2 metapaths = 128 PSUM rows)
NBLK = 8          # blocks per core
VIRTUAL_Z = 100

_PROGRAM_CACHE: dict = {}
LAST_RESULTS = None  # BassKernelResults of the most recent run (for test.py)
LAST_NC = None       # compiled program of the most recent run (for test.py)
LAST_IN_MAPS = None  # per-core input maps of the most recent run (for test.py)


def _build_program(T: int, CH: int):
    """T: node tiles per block (multiple of CH); CH: tiles per DMA chunk."""
    key = (T, CH)
    if key in _PROGRAM_CACHE:
        return _PROGRAM_CACHE[key]

    CPT = T // CH          # chunks per block
    NCH = NBLK * CPT       # chunks per core
    CHF = CH * D           # chunk free size
    TT = NBLK * T          # total tiles per core

    nc = bacc.Bacc("TRN2", target_bir_lowering=False, debug=False,
                   num_devices=N_CORES)
    xdat = nc.declare_dram_parameter("xdat", [NCH, 128, CHF], F32, isOutput=False)
    colp = nc.declare_dram_parameter("col", [128, TT], F32, isOutput=False)
    scalesp = nc.declare_dram_parameter("scales", [128, NBLK], F32, isOutput=False)
    iotap = nc.declare_dram_parameter("iota", [128, 128], F32, isOutput=False)
    identp = nc.declare_dram_parameter("ident", [128, 128], F32, isOutput=False)
    w1ap = nc.declare_dram_parameter("w1a", [128, A], F32, isOutput=False)
    w1bp = nc.declare_dram_parameter("w1b", [D - 128, A], F32, isOutput=False)
    b1p = nc.declare_dram_parameter("b1", [A, 1], F32, isOutput=False)
    qp = nc.declare_dram_parameter("q", [A, 1], F32, isOutput=False)
    resp = nc.declare_dram_parameter("res", [NBLK * GB, D], F32, isOutput=True)

    with tile.TileContext(nc) as tc:
        with tc.tile_pool(name="const", bufs=1) as cpool, \
             tc.tile_pool(name="chunks", bufs=3) as chpool, \
             tc.tile_pool(name="oh", bufs=6) as ohpool, \
             tc.tile_pool(name="small", bufs=1) as spool, \
             tc.tile_pool(name="xt", bufs=2) as xtpool, \
             tc.tile_pool(name="pm", bufs=2, space="PSUM") as pm, \
             tc.tile_pool(name="ptp", bufs=1, space="PSUM") as ptp, \
             tc.tile_pool(name="ph", bufs=1, space="PSUM") as ph, \
             tc.tile_pool(name="ps", bufs=1, space="PSUM") as ps, \
             tc.tile_pool(name="pout", bufs=2, space="PSUM") as pout, \
             tc.tile_pool(name="dram", bufs=1, space="DRAM") as dpool:

            # --- constants ---
            iota_t = cpool.tile([128, 128], F32, tag="iota")
            nc.scalar.dma_start(iota_t[:], iotap[:])
            ident_t = cpool.tile([128, 128], F32, tag="ident")
            nc.scalar.dma_start(ident_t[:], identp[:])
            w1a_t = cpool.tile([128, A], F32, tag="w1a")
            nc.scalar.dma_start(w1a_t[:], w1ap[:])
            w1b_t = cpool.tile([D - 128, A], F32, tag="w1b")
            nc.scalar.dma_start(w1b_t[:], w1bp[:])
            b1_t = cpool.tile([A, 1], F32, tag="b1")
            nc.scalar.dma_start(b1_t[:], b1p[:])
            q_t = cpool.tile([A, 1], F32, tag="q")
            nc.scalar.dma_start(q_t[:], qp[:])
            scales_t = cpool.tile([128, NBLK], F32, tag="scales")
            nc.scalar.dma_start(scales_t[:], scalesp[:])
            col_t = cpool.tile([128, TT], F32, tag="col")
            nc.scalar.dma_start(col_t[:], colp[:])
            means_all = cpool.tile([128, NBLK * D], F32, tag="means")
            scores_acc = cpool.tile([1, 128], F32, tag="sacc")

            eq = mybir.AluOpType.is_equal
            mult = mybir.AluOpType.mult

            # --- main streaming loop: masked segment sums per block ---
            for b in range(NBLK):
                psum_means = pm.tile([128, D], F32, tag="pmeans")
                for ci in range(CPT):
                    c = b * CPT + ci
                    chunk = chpool.tile([128, CHF], F32, tag="chunk")
                    nc.sync.dma_start(chunk[:], xdat[c])
                    for t in range(CH):
                        gt = b * T + ci * CH + t
                        oh = ohpool.tile([128, 128], F32, tag="oh")
                        nc.vector.tensor_scalar(out=oh[:], in0=iota_t[:],
                                                scalar1=col_t[:, gt:gt + 1],
                                                scalar2=None, op0=eq)
                        nc.tensor.matmul(psum_means[:], oh[:],
                                         chunk[:, t * D:(t + 1) * D],
                                         start=(ci == 0 and t == 0),
                                         stop=(ci == CPT - 1 and t == CH - 1))

                # --- block epilogue: means + attention scores ---
                msl = means_all[:, b * D:(b + 1) * D]
                nc.vector.tensor_scalar(out=msl, in0=psum_means[:],
                                        scalar1=scales_t[:, b:b + 1],
                                        scalar2=None, op0=mult)
                tp = ptp.tile([128, 256], F32, tag="tp")
                nc.tensor.transpose(tp[:, 0:128], means_all[:, b * D:b * D + 128],
                                    ident_t[:])
                nc.tensor.transpose(tp[0:22, 128:256],
                                    means_all[:, b * D + 128:b * D + 150],
                                    ident_t[:])
                xt = xtpool.tile([128, 256], F32, tag="xt")
                nc.scalar.copy(xt[:, 0:128], tp[:, 0:128])
                nc.scalar.copy(xt[0:22, 128:256], tp[0:22, 128:256])
                ph_t = ph.tile([128, 128], F32, tag="h")
                nc.tensor.matmul(ph_t[:], w1a_t[:], xt[:, 0:128],
                                 start=True, stop=False)
                nc.tensor.matmul(ph_t[:], w1b_t[:], xt[0:22, 128:256],
                                 start=False, stop=True)
                ht = xtpool.tile([128, 128], F32, tag="ht")
                nc.scalar.activation(ht[:], ph_t[:],
                                     mybir.ActivationFunctionType.Tanh,
                                     bias=b1_t[:, 0:1])
                ps_t = ps.tile([1, 128], F32, tag="s")
                nc.tensor.matmul(ps_t[:], q_t[:], ht[:], start=True, stop=True)
                if b == 0:
                    nc.vector.tensor_copy(scores_acc[:], ps_t[:])
                else:
                    nc.vector.tensor_add(scores_acc[:], scores_acc[:], ps_t[:])

            # --- global beta via 8-byte AllReduce + softmax ---
            s2 = spool.tile([1, 2], F32, tag="s2")
            nc.vector.reduce_sum(out=s2[0:1, 0:1], in_=scores_acc[0:1, 0:64],
                                 axis=mybir.AxisListType.X)
            nc.vector.reduce_sum(out=s2[0:1, 1:2], in_=scores_acc[0:1, 64:128],
                                 axis=mybir.AxisListType.X)
            cc_in = dpool.tile([1, 2], F32)
            cc_out = dpool.tile([1, 2], F32)
            nc.gpsimd.dma_start(cc_in[:], s2[:])
            nc.gpsimd.collective_compute(
                "AllReduce", mybir.AluOpType.add,
                replica_groups=[list(range(N_CORES))],
                ins=[cc_in.opt()], outs=[cc_out.opt()])
            sg = spool.tile([1, 2], F32, tag="sg")
            nc.gpsimd.dma_start(sg[:], cc_out[:])
            e = spool.tile([1, 2], F32, tag="e")
            nc.scalar.activation(e[:], sg[:], mybir.ActivationFunctionType.Exp,
                                 scale=1.0 / B)
            esum = spool.tile([1, 1], F32, tag="esum")
            nc.vector.reduce_sum(out=esum[:], in_=e[:], axis=mybir.AxisListType.X)
            erec = spool.tile([1, 1], F32, tag="erec")
            nc.vector.reciprocal(erec[:], esum[:])
            beta = spool.tile([1, 2], F32, tag="beta")
            nc.vector.tensor_scalar(out=beta[:], in0=e[:],
                                    scalar1=erec[0:1, 0:1], scalar2=None,
                                    op0=mult)
            ones_t = spool.tile([1, 128], F32, tag="ones")
            nc.vector.memset(ones_t[:], 1.0)
            pbb = ps.tile([128, 2], F32, tag="bb")
            nc.tensor.matmul(pbb[:], ones_t[:], beta[:], start=True, stop=True)
            beta_bc = spool.tile([128, 2], F32, tag="bbc")
            nc.scalar.copy(beta_bc[:], pbb[:])
            tmp1 = spool.tile([128, 64], F32, tag="tmp1")
            nc.vector.tensor_scalar(out=tmp1[:], in0=ident_t[:, 0:64],
                                    scalar1=beta_bc[:, 0:1], scalar2=None,
                                    op0=mult)
            tmp2 = spool.tile([128, 64], F32, tag="tmp2")
            nc.vector.tensor_scalar(out=tmp2[:], in0=ident_t[:, 64:128],
                                    scalar1=beta_bc[:, 1:2], scalar2=None,
                                    op0=mult)
            bsel = spool.tile([128, 64], F32, tag="bsel")
            nc.vector.tensor_add(bsel[:], tmp1[:], tmp2[:])

            # --- final combine + output ---
            for b in range(NBLK):
                po = pout.tile([64, D], F32, tag="po")
                nc.tensor.matmul(po[:], bsel[:], means_all[:, b * D:(b + 1) * D],
                                 start=True, stop=True)
                osb = xtpool.tile([64, D], F32, tag="osb")
                nc.scalar.copy(osb[:], po[:])
                nc.scalar.dma_start(resp[b * GB:(b + 1) * GB, :], osb[:])

    nc.compile()
    _PROGRAM_CACHE[key] = nc
    return nc


def kernel(out, z, batch, W1, b1, q, num_graphs):
    global LAST_RESULTS
    out = np.ascontiguousarray(np.asarray(out, dtype=np.float32))
    z = np.asarray(z).astype(np.int64)
    batch = np.asarray(batch).astype(np.int64)
    W1 = np.asarray(W1, dtype=np.float32)
    b1 = np.asarray(b1, dtype=np.float32)
    q = np.asarray(q, dtype=np.float32)
    assert int(num_graphs) == B
    N = out.shape[0]
    assert out.shape[1] == D and W1.shape == (D, A)

    # --- shard boundaries: 64 blocks of GB graphs, cut at graph boundaries ---
    cuts = np.searchsorted(batch, np.arange(0, B + 1, GB))
    nb_per_block = np.diff(cuts)
    T = int(np.ceil(nb_per_block.max() / 128.0))
    T = max(4, ((T + 3) // 4) * 4)
    CH = T // 4
    CPT = 4
    NCH = NBLK * CPT
    CHF = CH * D
    TT = NBLK * T

    # --- per-(graph, metapath) reciprocal counts ---
    keyv = 2 * batch + (z == VIRTUAL_Z)
    cnt = np.bincount(keyv, minlength=2 * B).reshape(B, 2).astype(np.float32)
    rcnt = 1.0 / np.maximum(cnt, 1.0)                       # [B, 2]

    nvirt = (z == VIRTUAL_Z).astype(np.float32)

    iota = np.tile(np.arange(128, dtype=np.float32), (128, 1))
    ident = np.eye(128, dtype=np.float32)
    w1a = np.ascontiguousarray(W1[:128])
    w1b = np.ascontiguousarray(W1[128:])
    b1r = np.ascontiguousarray(b1.reshape(A, 1))
    qr = np.ascontiguousarray(q.reshape(A, 1))

    in_maps = []
    for core in range(N_CORES):
        arr = np.zeros((NBLK * T * 128, D), dtype=np.float32)
        colv = np.full(NBLK * T * 128, -1.0, dtype=np.float32)
        for j in range(NBLK):
            k = NBLK * core + j
            lo, hi = int(cuts[k]), int(cuts[k + 1])
            nb = hi - lo
            base = j * T * 128
            arr[base:base + nb] = out[lo:hi]
            colv[base:base + nb] = ((batch[lo:hi] - k * GB)
                                    + GB * nvirt[lo:hi]).astype(np.float32)
        # chunk-permuted layout: [NCH, 128, CH*D]
        xarr = np.ascontiguousarray(
            arr.reshape(NCH, CH, 128, D).transpose(0, 2, 1, 3)
        ).reshape(NCH, 128, CHF)
        colarr = np.ascontiguousarray(colv.reshape(TT, 128).T)
        # scales rows: j<64 -> (graph j, real), j>=64 -> (graph j-64, virtual)
        g0 = core * NBLK * GB
        sc = np.empty((128, NBLK), dtype=np.float32)
        for j in range(NBLK):
            gids = g0 + j * GB + np.arange(GB)
            sc[0:64, j] = rcnt[gids, 0]
            sc[64:128, j] = rcnt[gids, 1]
        in_maps.append({
            "xdat": xarr, "col": colarr, "scales": sc,
            "iota": iota, "ident": ident,
            "w1a": w1a, "w1b": w1b, "b1": b1r, "q": qr,
        })

    nc = _build_program(T, CH)
    global LAST_NC, LAST_IN_MAPS
    LAST_NC, LAST_IN_MAPS = nc, in_maps
    res = run_bass_kernel_spmd(nc, in_maps, core_ids=list(range(N_CORES)))
    LAST_RESULTS = res
    outp = np.concatenate([res.results[i]["res"] for i in range(N_CORES)], axis=0)
    return outp.astype(np.float32)

